# revision 16
# baseline (speedup 1.0000x reference)
"""Trainium2 Bass kernel for nn_EntRelJointDecoder.

Computes (loss, joint_score, q_score) of the reference EntRelJointDecoder
module, sharded over 8 NeuronCores by the x (head-token) axis.

Device strategy (per core, x-chunk of 14 per batch):
  - pair   = gelu(h_part[x] + t_part[y] + b)  built as [i, (b,x,y)] tiles
  - q      = pair.T @ vU   with fp32r matmuls, PSUM out [rows=(x,y), (z,o)]
  - joint  = pair.T @ final_w + final_b (extra K=1 ones matmul for the bias)
  - Stage exp(q) / exp(joint) to DRAM; host recovers q = ln(E) exactly.
    This removes the PSUM->SBUF f32 copy pass entirely (exp *is* the evict).
  - Loss pipeline on device: s = sum_o exp(q), rs = 1/s via exp(-ln s),
    P = E * rs (softmax probs, gpsimd), X = exp(P), zs = sum_o X,
    lz = ln zs;  per-row partial sums of  maskq*lz  and  maskq*P[label]
    (one-hot selq from host, fused multiply+accumulate via
    scalar_tensor_tensor).  Element loss handled analogously from
    exp(joint) staged in SBUF.  Host combines partials into the scalar loss.
"""

import sys

sys.path.insert(0, "/opt/trn_rl_repo")

from contextlib import ExitStack

import numpy as np
import ml_dtypes

import concourse.bass as bass
import concourse.tile as tile
from concourse import bacc, mybir
from concourse.bass_utils import run_bass_kernel_spmd


def _patched_act_tables(orig):
    """Pin Exp and Ln to the one table set containing both, so the
    per-tile Exp/Ln interleave doesn't thrash ACT_TABLE_LOADs."""
    import functools

    @functools.cache
    def wrapper(arch):
        tabs = {k: set(v) for k, v in orig(arch).items()}
        AFt = mybir.ActivationFunctionType
        for name, funcs in tabs.items():
            if name != "natural_log_exp_and_others":
                funcs.discard(AFt.Exp)
                funcs.discard(AFt.Ln)
        return tabs

    return wrapper


bacc.get_activation_tables = _patched_act_tables(bacc.get_activation_tables)

F32 = mybir.dt.float32
F32R = mybir.dt.float32r
BF16 = mybir.dt.bfloat16
F16 = mybir.dt.float16
U8 = mybir.dt.uint8
AF = mybir.ActivationFunctionType
ALU = mybir.AluOpType
AXL = mybir.AxisListType

B, S, H, I, L, O = 2, 112, 768, 256, 54, 10
NC = 8
XC = S // NC            # 14 x's per core per batch
RPB = XC * S            # 1568 rows per batch per core
ROWS = B * RPB          # 3136 rows per core
HC = H // 128           # 6 h-chunks
IC = I // 128           # 2 i-chunks

# row tiles: per batch 12 x 128 + 1 x 32
TILES = []
for b in range(B):
    off = 0
    while off < RPB:
        nr = min(128, RPB - off)
        TILES.append((b, off, nr))
        off += nr
NT = len(TILES)         # 26

# q-matmul free-dim chunks (z-ranges) placed at bank-aligned PSUM offsets
ZCH = [(0, 51, 0), (51, 51, 512), (102, 10, 1024)]   # (z0, nz, psum_col)
JCOL = 1124             # joint block inside PSUM bank 2
PSW = JCOL + L          # 1178


def _build_program():
    nc = bacc.Bacc(target_bir_lowering=False)

    def par(name, shape, dt, out=False):
        return nc.declare_dram_parameter(name, list(shape), dt, isOutput=out)

    xt_d = par("xt", [128, HC * 2 * S], F32)          # xT  [h, (hc,b,s)]
    hxt_d = par("hxt", [128, HC * B * XC], F32)       # xT cols of this core's x's
    wh_d = par("wh", [128, HC * I], F32)
    wt_d = par("wt", [128, HC * I], F32)
    wv_d = par("wv", [128, HC * I], F32)
    ut_d = par("ut", [128, O * IC * I], F32R)         # U^T [j, (o,jc,i)]
    wf_d = par("wf", [128, IC * L], F32R)
    fb_d = par("fb", [1, L], F32)
    pb_d = par("pb", [128, IC], F32)
    vb_d = par("vb", [128, IC], F32)
    selq_d = par("selq", [128, NT * S * O], U8)
    maskq_d = par("maskq", [128, NT * S], BF16)
    selj_d = par("selj", [128, NT * L], U8)
    maskj_d = par("maskj", [128, NT], BF16)

    eq_d = par("eq", [ROWS, S * O], F32, out=True)    # exp(q) staged
    ej_d = par("ej", [128, NT * L], F32, out=True)    # exp(joint) staged
    part_d = par("partials", [128, 3 * NT], F32, out=True)

    with tile.TileContext(nc) as tc, ExitStack() as ctx:
        cst = ctx.enter_context(tc.tile_pool(name="cst", bufs=1))

        def load(dram, shape, dt, name):
            t = cst.tile(shape, dt, name=name)
            # narrow-dtype loads corrupt on the sync HWDGE queue; use ACT's
            eng = nc.scalar if mybir.dt.size(dt) < 4 else nc.sync
            eng.dma_start(t[:], dram[:])
            return t

        xt = load(xt_d, [128, HC * 2 * S], F32, "xt_t")
        hxt = load(hxt_d, [128, HC * B * XC], F32, "hxt_t")
        wh = load(wh_d, [128, HC * I], F32, "wh_t")
        wt = load(wt_d, [128, HC * I], F32, "wt_t")
        wv = load(wv_d, [128, HC * I], F32, "wv_t")
        ut = load(ut_d, [128, O * IC * I], F32R, "ut_t")
        wf = load(wf_d, [128, IC * L], F32R, "wf_t")
        fb = load(fb_d, [1, L], F32, "fb_t")
        pb = load(pb_d, [128, IC], F32, "pb_t")
        vb = load(vb_d, [128, IC], F32, "vb_t")
        maskq = load(maskq_d, [128, NT * S], BF16, "maskq_t")
        selj = load(selj_d, [128, NT * L], U8, "selj_t")
        maskj = load(maskj_d, [128, NT], BF16, "maskj_t")

        ones = cst.tile([1, 128], F32)
        nc.vector.memset(ones[:], 1.0)

        strip_lz = cst.tile([128, NT], F32)
        strip_pl = cst.tile([128, NT], F32)
        nc.vector.memset(strip_lz[:], 0.0)
        nc.vector.memset(strip_pl[:], 0.0)

        # garbage rows of partial tiles must stay finite through ln()
        ej_all = cst.tile([128, NT * L], F32)
        nc.vector.memset(ej_all[:], 1.0)

        t_sb = [cst.tile([128, 2 * S], F32, name=f"t_sb{k}") for k in range(IC)]
        val_sb = [cst.tile([128, 2 * S], F32R, name=f"val_sb{k}") for k in range(IC)]
        hb_sb = [cst.tile([128, B * XC], F32, name=f"hb_sb{k}") for k in range(IC)]
        vu_sb = [[cst.tile([128, O * S], F32R, name=f"vu{b}{k}") for k in range(IC)] for b in range(B)]
        pair_sb = [cst.tile([128, ROWS], F32R, name=f"pair{k}") for k in range(IC)]

        # ---- prologue: t_part / value / h-cols -------------------------------
        with tc.tile_pool(name="psA", bufs=2, space="PSUM") as psA, \
             tc.tile_pool(name="tmpA", bufs=2) as tmpA:
            for k in range(IC):
                tp = psA.tile([128, 2 * S], F32, tag="p224")
                for hc in range(HC):
                    nc.tensor.matmul(
                        tp[:], wt[:, hc * I + k * 128: hc * I + k * 128 + 128],
                        xt[:, hc * 2 * S: (hc + 1) * 2 * S],
                        start=(hc == 0), stop=(hc == HC - 1))
                nc.vector.tensor_copy(t_sb[k][:], tp[:])

                vp = psA.tile([128, 2 * S], F32, tag="p224")
                for hc in range(HC):
                    nc.tensor.matmul(
                        vp[:], wv[:, hc * I + k * 128: hc * I + k * 128 + 128],
                        xt[:, hc * 2 * S: (hc + 1) * 2 * S],
                        start=(hc == 0), stop=(hc == HC - 1))
                nc.scalar.activation(val_sb[k][:], vp[:], AF.Gelu,
                                     bias=vb[:, k: k + 1])

                hp = psA.tile([128, B * XC], F32, tag="p28")
                for hc in range(HC):
                    nc.tensor.matmul(
                        hp[:], wh[:, hc * I + k * 128: hc * I + k * 128 + 128],
                        hxt[:, hc * B * XC: (hc + 1) * B * XC],
                        start=(hc == 0), stop=(hc == HC - 1))
                nc.scalar.activation(hb_sb[k][:], hp[:], AF.Identity,
                                     bias=pb[:, k: k + 1])

            # ---- pair = gelu(t + h + b) in [i, (b,x,y)] layout --------------
            for k in range(IC):
                for b in range(B):
                    for g in range(2):
                        pre = tmpA.tile([128, 7 * S], F32, tag="pre")
                        t_b = t_sb[k][:, b * S: (b + 1) * S] \
                            .unsqueeze(1).broadcast_to([128, 7, S])
                        h_b = hb_sb[k][:, b * XC + g * 7: b * XC + g * 7 + 7] \
                            .unsqueeze(2).broadcast_to([128, 7, S])
                        nc.vector.tensor_add(
                            pre[:].rearrange("p (x y) -> p x y", y=S), t_b, h_b)
                        c0 = b * RPB + g * 7 * S
                        nc.scalar.activation(pair_sb[k][:, c0: c0 + 7 * S],
                                             pre[:], AF.Gelu)

        # ---- vU = U^T @ value  -> [i, (o,z)] per batch ----------------------
        with tc.tile_pool(name="psB", bufs=2, space="PSUM") as psB:
            for k in range(IC):
                for og in range(2):
                    vp = psB.tile([128, 5 * 256], F32, tag="vu")
                    for oi in range(5):
                        o = og * 5 + oi
                        for jc in range(IC):
                            nc.tensor.matmul(
                                vp[:, oi * 256: oi * 256 + 2 * S],
                                ut[:, o * IC * I + jc * I + k * 128:
                                   o * IC * I + jc * I + k * 128 + 128],
                                val_sb[jc][:],
                                start=(jc == 0), stop=(jc == IC - 1))
                    v3 = vp[:].rearrange("p (o bz) -> p o bz", o=5)[:, :, : 2 * S]
                    for b in range(B):
                        dst = vu_sb[b][k][:, og * 5 * S: (og + 1) * 5 * S]
                        dst3 = dst.rearrange("p (o z) -> p o z", o=5)
                        src3 = v3[:, :, b * S: (b + 1) * S]
                        if b == 0:
                            nc.vector.tensor_copy(dst3, src3)
                        else:
                            nc.scalar.copy(dst3, src3)

        # ---- main loop ------------------------------------------------------
        with tc.tile_pool(name="psQ", bufs=2, space="PSUM") as psQ, \
             tc.tile_pool(name="big", bufs=3) as big, \
             tc.tile_pool(name="med", bufs=2) as med, \
             tc.tile_pool(name="sml", bufs=3) as sml:
            for t, (b, r0, nr) in enumerate(TILES):
                g0 = b * RPB + r0
                ps = psQ.tile([128, PSW], F32, tag="q")
                lhsTs = [pair_sb[k][:, g0: g0 + nr] for k in range(IC)]
                v2s = [vu_sb[b][k][:].rearrange("p (o z) -> p z o", o=O)
                       for k in range(IC)]
                for (z0, nz, pc) in ZCH:
                    for k in range(IC):
                        nc.tensor.matmul(ps[:nr, pc: pc + nz * O], lhsTs[k],
                                         v2s[k][:, z0: z0 + nz, :],
                                         start=(k == 0), stop=(k == IC - 1))
                for k in range(IC):
                    nc.tensor.matmul(ps[:nr, JCOL: JCOL + L], lhsTs[k],
                                     wf[:, k * L: (k + 1) * L],
                                     start=(k == 0), stop=False)
                nc.tensor.matmul(ps[:nr, JCOL: JCOL + L], ones[:, :nr], fb[:],
                                 start=False, stop=True)

                # E = exp(q) staged (two PSUM segments -> contiguous SBUF)
                E = big.tile([128, S * O], F32, tag="E")
                nc.scalar.activation(
                    E[:nr, 0:1020].rearrange("p (g c) -> p g c", c=510),
                    ps[:nr, 0:1024].rearrange("p (g c) -> p g c", c=512)[:, :, 0:510],
                    AF.Exp)
                nc.scalar.activation(E[:nr, 1020:1120], ps[:nr, 1024:1124], AF.Exp)
                # EJ = exp(joint)
                nc.scalar.activation(ej_all[:nr, t * L: (t + 1) * L],
                                     ps[:nr, JCOL: JCOL + L], AF.Exp)

                # s = sum_o E ; rs = 1/s = exp(-ln s)
                s = sml.tile([128, S], F32, tag="s")
                nc.vector.tensor_reduce(
                    s[:nr, 0:102],
                    E[:nr, 0:1020].rearrange("p (z o) -> p z o", o=O),
                    axis=AXL.X, op=ALU.add)
                nc.vector.tensor_reduce(
                    s[:nr, 102:112],
                    E[:nr, 1020:1120].rearrange("p (z o) -> p z o", o=O),
                    axis=AXL.X, op=ALU.add)
                ls = sml.tile([128, S], F32, tag="ls")
                nc.scalar.activation(ls[:nr], s[:nr], AF.Ln)
                rs = sml.tile([128, S], F32, tag="rs")
                nc.scalar.activation(rs[:nr], ls[:nr], AF.Exp, scale=-1.0)

                # P = E * rs (softmax probs)
                P = big.tile([128, S * O], F16, tag="P")
                nc.gpsimd.tensor_mul(
                    P[:nr].rearrange("p (z o) -> p z o", o=O),
                    E[:nr].rearrange("p (z o) -> p z o", o=O),
                    rs[:nr].unsqueeze(2).broadcast_to([nr, S, O]))

                # X = exp(P); zs = sum_o X; lz = ln zs
                X = big.tile([128, S * O], F32, tag="X")
                nc.scalar.activation(X[:nr], P[:nr], AF.Exp)
                zs = sml.tile([128, S], F32, tag="zs")
                nc.vector.tensor_reduce(
                    zs[:nr], X[:nr].rearrange("p (z o) -> p z o", o=O),
                    axis=AXL.X, op=ALU.add)
                lz = sml.tile([128, S], F32, tag="lz")
                nc.scalar.activation(lz[:nr], zs[:nr], AF.Ln)

                # partial sums
                selq_t = big.tile([128, S * O], U8, tag="selq")
                nc.scalar.dma_start(selq_t[:],
                                    selq_d[:, t * S * O: (t + 1) * S * O])
                scr_b = big.tile([128, S * O], BF16, tag="scrb")
                nc.vector.scalar_tensor_tensor(
                    out=scr_b[:nr], in0=P[:nr], scalar=1.0, in1=selq_t[:nr],
                    op0=ALU.mult, op1=ALU.mult,
                    accum_out=strip_pl[:nr, t: t + 1])
                scr_s = sml.tile([128, S], BF16, tag="scrs")
                nc.vector.scalar_tensor_tensor(
                    out=scr_s[:nr], in0=lz[:nr], scalar=1.0,
                    in1=maskq[:nr, t * S: (t + 1) * S],
                    op0=ALU.mult, op1=ALU.mult,
                    accum_out=strip_lz[:nr, t: t + 1])

                # stage exp(q) out (one contiguous transfer per tile)
                deng = nc.sync if t % 2 == 0 else nc.scalar
                deng.dma_start(eq_d[g0: g0 + nr, :], E[:nr, :])

            # ---- joint (element) loss, batched over all tiles ---------------
            sj = med.tile([128, NT], F32, tag="sj")
            nc.vector.tensor_reduce(
                sj[:], ej_all[:].rearrange("p (t l) -> p t l", l=L),
                axis=AXL.X, op=ALU.add)
            lsj = med.tile([128, NT], F32, tag="lsj")
            nc.scalar.activation(lsj[:], sj[:], AF.Ln)
            tmpj = med.tile([128, NT * L], BF16, tag="tmpj")
            ejl = med.tile([128, NT], F32, tag="ejl")
            nc.vector.tensor_mul(tmpj[:], ej_all[:], selj[:])
            nc.vector.tensor_reduce(
                ejl[:], tmpj[:].rearrange("p (t l) -> p t l", l=L),
                axis=AXL.X, op=ALU.add)
            # guard: masked-out rows have selj == 0 -> ejl == 0 -> ln(0);
            # clamp with max(ejl, tiny) to keep ln finite (result masked later)
            ejl2 = med.tile([128, NT], F32, tag="ejl2")
            nc.vector.tensor_scalar_max(ejl2[:], ejl[:], 1e-30)
            lnjl = med.tile([128, NT], F32, tag="lnjl")
            nc.scalar.activation(lnjl[:], ejl2[:], AF.Ln)
            dj = med.tile([128, NT], F32, tag="dj")
            nc.vector.tensor_sub(dj[:], lsj[:], lnjl[:])
            djm = med.tile([128, NT], F32, tag="djm")
            nc.vector.tensor_mul(djm[:], dj[:], maskj[:])

            nc.sync.dma_start(ej_d[:], ej_all[:])
            nc.sync.dma_start(part_d[:, 0:NT], strip_lz[:])
            nc.sync.dma_start(part_d[:, NT: 2 * NT], strip_pl[:])
            nc.sync.dma_start(part_d[:, 2 * NT: 3 * NT], djm[:])

    nc.finalize()
    return nc


_NC_CACHE = None


def _get_program():
    global _NC_CACHE
    if _NC_CACHE is None:
        _NC_CACHE = _build_program()
    return _NC_CACHE


def _pack_inputs(seq, jl, jlm, qm, qmm, pair_w, pair_b, final_w, final_b,
                 value_w, value_b, U):
    """Build the 8 per-core input maps (all host-side numpy)."""
    xT = seq.transpose(2, 0, 1).reshape(H, 2 * S)          # [h, (b,s)]
    xt_h = xT.reshape(HC, 128, 2 * S).transpose(1, 0, 2).reshape(128, HC * 2 * S)
    xt_h = np.ascontiguousarray(xt_h, dtype=np.float32)

    def wpack(w):  # [H, I] -> [128, (hc, i)]
        return np.ascontiguousarray(
            w.reshape(HC, 128, I).transpose(1, 0, 2).reshape(128, HC * I),
            dtype=np.float32)

    wh_h = wpack(pair_w[:H])
    wt_h = wpack(pair_w[H:])
    wv_h = wpack(value_w)
    # U^T: [o, j, i] with j on partitions -> [128, (o, jc, i)]
    ut_h = np.ascontiguousarray(
        U.transpose(0, 2, 1).reshape(O, IC, 128, I).transpose(2, 0, 1, 3)
        .reshape(128, O * IC * I), dtype=np.float32)
    wf_h = np.ascontiguousarray(
        final_w.reshape(IC, 128, L).transpose(1, 0, 2).reshape(128, IC * L),
        dtype=np.float32)
    fb_h = np.ascontiguousarray(final_b.reshape(1, L), dtype=np.float32)
    pb_h = np.ascontiguousarray(pair_b.reshape(IC, 128).T, dtype=np.float32)
    vb_h = np.ascontiguousarray(value_b.reshape(IC, 128).T, dtype=np.float32)

    jl = jl.astype(np.int32)
    qm = qm.astype(np.int32)
    jlm_f = jlm.astype(np.float32)
    qmm_f = qmm.astype(np.float32)

    in_maps = []
    for c in range(NC):
        x0 = c * XC
        hxt_h = np.ascontiguousarray(
            xT.reshape(HC, 128, B, S)[:, :, :, x0: x0 + XC]
            .transpose(1, 0, 2, 3).reshape(128, HC * B * XC), dtype=np.float32)

        # per-row tensors in tile layout [128 partitions, NT * inner]
        selq_h = np.zeros((128, NT, S, O), dtype=np.uint8)
        maskq_h = np.zeros((128, NT, S), dtype=np.float32)
        selj_h = np.zeros((128, NT, L), dtype=np.uint8)
        maskj_h = np.zeros((128, NT), dtype=np.float32)
        for t, (b, r0, nr) in enumerate(TILES):
            rows = np.arange(r0, r0 + nr)
            xl = rows // S
            y = rows % S
            xg = x0 + xl
            lbl_q = qm[b, xg, y]                       # [nr, S] int  (z axis)
            msk_q = qmm_f[b, xg, y]                    # [nr, S]
            onehot = (lbl_q[:, :, None] == np.arange(O)[None, None, :])
            selq_h[:nr, t] = (onehot & (msk_q[:, :, None] > 0)).astype(np.uint8)
            maskq_h[:nr, t] = msk_q
            lbl_j = jl[b, xg, y]                       # [nr] int
            msk_j = jlm_f[b, xg, y]                    # [nr]
            oh_j = (lbl_j[:, None] == np.arange(L)[None, :])
            selj_h[:nr, t] = (oh_j & (msk_j[:, None] > 0)).astype(np.uint8)
            maskj_h[:nr, t] = msk_j

        in_maps.append(dict(
            xt=xt_h, hxt=hxt_h, wh=wh_h, wt=wt_h, wv=wv_h, ut=ut_h, wf=wf_h,
            fb=fb_h, pb=pb_h, vb=vb_h,
            selq=selq_h.reshape(128, NT * S * O),
            maskq=maskq_h.reshape(128, NT * S).astype(ml_dtypes.bfloat16),
            selj=selj_h.reshape(128, NT * L),
            maskj=maskj_h.astype(ml_dtypes.bfloat16),
        ))
    return in_maps


def kernel(seq_encoder_reprs, joint_label_matrix, joint_label_matrix_mask,
           quintuplet_matrix, quintuplet_matrix_mask,
           pair_w, pair_b, final_w, final_b, value_w, value_b, U,
           _want_trace=False):
    seq = np.asarray(seq_encoder_reprs, dtype=np.float32)
    in_maps = _pack_inputs(seq, np.asarray(joint_label_matrix),
                           np.asarray(joint_label_matrix_mask),
                           np.asarray(quintuplet_matrix),
                           np.asarray(quintuplet_matrix_mask),
                           np.asarray(pair_w, dtype=np.float32),
                           np.asarray(pair_b, dtype=np.float32),
                           np.asarray(final_w, dtype=np.float32),
                           np.asarray(final_b, dtype=np.float32),
                           np.asarray(value_w, dtype=np.float32),
                           np.asarray(value_b, dtype=np.float32),
                           np.asarray(U, dtype=np.float32))

    nc = _get_program()
    res = run_bass_kernel_spmd(nc, in_maps, list(range(NC)),
                               trace=_want_trace)
    kernel._last_results = res

    q_score = np.empty((B, S, S, S, O), dtype=np.float32)
    joint_score = np.empty((B, S, S, L), dtype=np.float32)
    q_sum = 0.0
    el_sum = 0.0
    for c in range(NC):
        r = res.results[c]
        x0 = c * XC
        eq = r["eq"].reshape(B, XC, S, S, O)
        q_score[:, x0: x0 + XC] = np.log(eq)
        ej = r["ej"]                                  # [128, NT*L]
        ejr = ej.reshape(128, NT, L)
        for t, (b, r0, nr) in enumerate(TILES):
            rows = np.arange(r0, r0 + nr)
            joint_score[b, x0 + rows // S, rows % S] = np.log(ejr[:nr, t])
        p = r["partials"].astype(np.float64)
        q_sum += p[:, 0:NT].sum() - p[:, NT:2 * NT].sum()
        el_sum += p[:, 2 * NT:3 * NT].sum()

    q_cnt = max(float(np.asarray(quintuplet_matrix_mask).sum()), 1.0)
    el_cnt = max(float(np.asarray(joint_label_matrix_mask).sum()), 1.0)
    loss = np.float32(el_sum / el_cnt + q_sum / q_cnt)
    return loss, joint_score, q_score


kernel._last_exec_ns = None


# revision 17
# speedup vs baseline: 1.0057x; 1.0057x over previous
"""Trainium2 Bass kernel for nn_EntRelJointDecoder.

Computes (loss, joint_score, q_score) of the reference EntRelJointDecoder
module, sharded over 8 NeuronCores by the x (head-token) axis.

Device strategy (per core, x-chunk of 14 per batch):
  - pair   = gelu(h_part[x] + t_part[y] + b)  built as [i, (b,x,y)] tiles
  - q      = pair.T @ vU   with fp32r matmuls, PSUM out [rows=(x,y), (z,o)]
  - joint  = pair.T @ final_w + final_b (extra K=1 ones matmul for the bias)
  - Stage exp(q) / exp(joint) to DRAM; host recovers q = ln(E) exactly.
    This removes the PSUM->SBUF f32 copy pass entirely (exp *is* the evict).
  - Loss pipeline on device: s = sum_o exp(q), rs = 1/s via exp(-ln s),
    P = E * rs (softmax probs, gpsimd), X = exp(P), zs = sum_o X,
    lz = ln zs;  per-row partial sums of  maskq*lz  and  maskq*P[label]
    (one-hot selq from host, fused multiply+accumulate via
    scalar_tensor_tensor).  Element loss handled analogously from
    exp(joint) staged in SBUF.  Host combines partials into the scalar loss.
"""

import sys

sys.path.insert(0, "/opt/trn_rl_repo")

from contextlib import ExitStack

import numpy as np
import ml_dtypes

import concourse.bass as bass
import concourse.tile as tile
from concourse import bacc, mybir
from concourse.bass_utils import run_bass_kernel_spmd


def _patched_act_tables(orig):
    """Pin Exp and Ln to the one table set containing both, so the
    per-tile Exp/Ln interleave doesn't thrash ACT_TABLE_LOADs."""
    import functools

    @functools.cache
    def wrapper(arch):
        tabs = {k: set(v) for k, v in orig(arch).items()}
        AFt = mybir.ActivationFunctionType
        for name, funcs in tabs.items():
            if name != "natural_log_exp_and_others":
                funcs.discard(AFt.Exp)
                funcs.discard(AFt.Ln)
        return tabs

    return wrapper


bacc.get_activation_tables = _patched_act_tables(bacc.get_activation_tables)

F32 = mybir.dt.float32
F32R = mybir.dt.float32r
BF16 = mybir.dt.bfloat16
F16 = mybir.dt.float16
U8 = mybir.dt.uint8
AF = mybir.ActivationFunctionType
ALU = mybir.AluOpType
AXL = mybir.AxisListType

B, S, H, I, L, O = 2, 112, 768, 256, 54, 10
NC = 8
XC = S // NC            # 14 x's per core per batch
RPB = XC * S            # 1568 rows per batch per core
ROWS = B * RPB          # 3136 rows per core
HC = H // 128           # 6 h-chunks
IC = I // 128           # 2 i-chunks

# row tiles: per batch 12 x 128 + 1 x 32
TILES = []
for b in range(B):
    off = 0
    while off < RPB:
        nr = min(128, RPB - off)
        TILES.append((b, off, nr))
        off += nr
NT = len(TILES)         # 26

# q-matmul free-dim chunks (z-ranges) placed at bank-aligned PSUM offsets
ZCH = [(0, 51, 0), (51, 51, 512), (102, 10, 1024)]   # (z0, nz, psum_col)
JCOL = 1124             # joint block inside PSUM bank 2
PSW = JCOL + L          # 1178


def _build_program():
    nc = bacc.Bacc(target_bir_lowering=False)

    def par(name, shape, dt, out=False):
        return nc.declare_dram_parameter(name, list(shape), dt, isOutput=out)

    xt_d = par("xt", [128, HC * 2 * S], F32)          # xT  [h, (hc,b,s)]
    hxt_d = par("hxt", [128, HC * B * XC], F32)       # xT cols of this core's x's
    wh_d = par("wh", [128, HC * I], F32)
    wt_d = par("wt", [128, HC * I], F32)
    wv_d = par("wv", [128, HC * I], F32)
    ut_d = par("ut", [128, O * IC * I], F32R)         # U^T [j, (o,jc,i)]
    wf_d = par("wf", [128, IC * L], F32R)
    fb_d = par("fb", [1, L], F32)
    pb_d = par("pb", [128, IC], F32)
    vb_d = par("vb", [128, IC], F32)
    selq_d = par("selq", [128, NT * S * O], U8)
    maskq_d = par("maskq", [128, NT * S], BF16)
    selj_d = par("selj", [128, NT * L], U8)
    maskj_d = par("maskj", [128, NT], BF16)

    eq_d = par("eq", [ROWS, S * O], F32, out=True)    # exp(q) staged
    ej_d = par("ej", [128, NT * L], F32, out=True)    # exp(joint) staged
    part_d = par("partials", [128, 3 * NT], F32, out=True)

    with tile.TileContext(nc) as tc, ExitStack() as ctx:
        cst = ctx.enter_context(tc.tile_pool(name="cst", bufs=1))

        def load(dram, shape, dt, name):
            t = cst.tile(shape, dt, name=name)
            # narrow-dtype loads corrupt on the sync HWDGE queue; use ACT's
            eng = nc.scalar if mybir.dt.size(dt) < 4 else nc.sync
            eng.dma_start(t[:], dram[:])
            return t

        xt = load(xt_d, [128, HC * 2 * S], F32, "xt_t")
        hxt = load(hxt_d, [128, HC * B * XC], F32, "hxt_t")
        wh = load(wh_d, [128, HC * I], F32, "wh_t")
        wt = load(wt_d, [128, HC * I], F32, "wt_t")
        wv = load(wv_d, [128, HC * I], F32, "wv_t")
        ut = load(ut_d, [128, O * IC * I], F32R, "ut_t")
        wf = load(wf_d, [128, IC * L], F32R, "wf_t")
        fb = load(fb_d, [1, L], F32, "fb_t")
        pb = load(pb_d, [128, IC], F32, "pb_t")
        vb = load(vb_d, [128, IC], F32, "vb_t")
        maskq = load(maskq_d, [128, NT * S], BF16, "maskq_t")
        selj = load(selj_d, [128, NT * L], U8, "selj_t")
        maskj = load(maskj_d, [128, NT], BF16, "maskj_t")

        ones = cst.tile([1, 128], F32)
        nc.vector.memset(ones[:], 1.0)

        strip_lz = cst.tile([128, NT], F32)
        strip_pl = cst.tile([128, NT], F32)
        nc.vector.memset(strip_lz[:], 0.0)
        nc.vector.memset(strip_pl[:], 0.0)

        # garbage rows of partial tiles must stay finite through ln()
        ej_all = cst.tile([128, NT * L], F32)
        nc.vector.memset(ej_all[:], 1.0)

        t_sb = [cst.tile([128, 2 * S], F32, name=f"t_sb{k}") for k in range(IC)]
        val_sb = [cst.tile([128, 2 * S], F32R, name=f"val_sb{k}") for k in range(IC)]
        hb_sb = [cst.tile([128, B * XC], F32, name=f"hb_sb{k}") for k in range(IC)]
        vu_sb = [[cst.tile([128, O * S], F32R, name=f"vu{b}{k}") for k in range(IC)] for b in range(B)]
        pair_sb = [cst.tile([128, ROWS], F32R, name=f"pair{k}") for k in range(IC)]

        # ---- prologue: t_part / value / h-cols -------------------------------
        with tc.tile_pool(name="psA", bufs=2, space="PSUM") as psA, \
             tc.tile_pool(name="tmpA", bufs=2) as tmpA:
            for k in range(IC):
                tp = psA.tile([128, 2 * S], F32, tag="p224")
                for hc in range(HC):
                    nc.tensor.matmul(
                        tp[:], wt[:, hc * I + k * 128: hc * I + k * 128 + 128],
                        xt[:, hc * 2 * S: (hc + 1) * 2 * S],
                        start=(hc == 0), stop=(hc == HC - 1))
                nc.vector.tensor_copy(t_sb[k][:], tp[:])

                vp = psA.tile([128, 2 * S], F32, tag="p224")
                for hc in range(HC):
                    nc.tensor.matmul(
                        vp[:], wv[:, hc * I + k * 128: hc * I + k * 128 + 128],
                        xt[:, hc * 2 * S: (hc + 1) * 2 * S],
                        start=(hc == 0), stop=(hc == HC - 1))
                nc.scalar.activation(val_sb[k][:], vp[:], AF.Gelu,
                                     bias=vb[:, k: k + 1])

                hp = psA.tile([128, B * XC], F32, tag="p28")
                for hc in range(HC):
                    nc.tensor.matmul(
                        hp[:], wh[:, hc * I + k * 128: hc * I + k * 128 + 128],
                        hxt[:, hc * B * XC: (hc + 1) * B * XC],
                        start=(hc == 0), stop=(hc == HC - 1))
                nc.scalar.activation(hb_sb[k][:], hp[:], AF.Identity,
                                     bias=pb[:, k: k + 1])

            # ---- pair = gelu(t + h + b) in [i, (b,x,y)] layout --------------
            for k in range(IC):
                for b in range(B):
                    for g in range(2):
                        pre = tmpA.tile([128, 7 * S], F32, tag="pre")
                        t_b = t_sb[k][:, b * S: (b + 1) * S] \
                            .unsqueeze(1).broadcast_to([128, 7, S])
                        h_b = hb_sb[k][:, b * XC + g * 7: b * XC + g * 7 + 7] \
                            .unsqueeze(2).broadcast_to([128, 7, S])
                        nc.vector.tensor_add(
                            pre[:].rearrange("p (x y) -> p x y", y=S), t_b, h_b)
                        c0 = b * RPB + g * 7 * S
                        nc.scalar.activation(pair_sb[k][:, c0: c0 + 7 * S],
                                             pre[:], AF.Gelu)

        # ---- vU = U^T @ value  -> [i, (o,z)] per batch ----------------------
        with tc.tile_pool(name="psB", bufs=2, space="PSUM") as psB:
            for k in range(IC):
                for og in range(2):
                    vp = psB.tile([128, 5 * 256], F32, tag="vu")
                    for oi in range(5):
                        o = og * 5 + oi
                        for jc in range(IC):
                            nc.tensor.matmul(
                                vp[:, oi * 256: oi * 256 + 2 * S],
                                ut[:, o * IC * I + jc * I + k * 128:
                                   o * IC * I + jc * I + k * 128 + 128],
                                val_sb[jc][:],
                                start=(jc == 0), stop=(jc == IC - 1))
                    v3 = vp[:].rearrange("p (o bz) -> p o bz", o=5)[:, :, : 2 * S]
                    for b in range(B):
                        dst = vu_sb[b][k][:, og * 5 * S: (og + 1) * 5 * S]
                        dst3 = dst.rearrange("p (o z) -> p o z", o=5)
                        src3 = v3[:, :, b * S: (b + 1) * S]
                        if b == 0:
                            nc.vector.tensor_copy(dst3, src3)
                        else:
                            nc.scalar.copy(dst3, src3)

        # ---- main loop ------------------------------------------------------
        with tc.tile_pool(name="psQ", bufs=2, space="PSUM") as psQ, \
             tc.tile_pool(name="big", bufs=5) as big, \
             tc.tile_pool(name="med", bufs=2) as med, \
             tc.tile_pool(name="sml", bufs=8) as sml:
            for t, (b, r0, nr) in enumerate(TILES):
                g0 = b * RPB + r0
                ps = psQ.tile([128, PSW], F32, tag="q")
                lhsTs = [pair_sb[k][:, g0: g0 + nr] for k in range(IC)]
                v2s = [vu_sb[b][k][:].rearrange("p (o z) -> p z o", o=O)
                       for k in range(IC)]
                for (z0, nz, pc) in ZCH:
                    for k in range(IC):
                        nc.tensor.matmul(ps[:nr, pc: pc + nz * O], lhsTs[k],
                                         v2s[k][:, z0: z0 + nz, :],
                                         start=(k == 0), stop=(k == IC - 1))
                for k in range(IC):
                    nc.tensor.matmul(ps[:nr, JCOL: JCOL + L], lhsTs[k],
                                     wf[:, k * L: (k + 1) * L],
                                     start=(k == 0), stop=False)
                nc.tensor.matmul(ps[:nr, JCOL: JCOL + L], ones[:, :nr], fb[:],
                                 start=False, stop=True)

                # E = exp(q) staged (two PSUM segments -> contiguous SBUF)
                E = big.tile([128, S * O], F32, tag="E")
                nc.scalar.activation(
                    E[:nr, 0:1020].rearrange("p (g c) -> p g c", c=510),
                    ps[:nr, 0:1024].rearrange("p (g c) -> p g c", c=512)[:, :, 0:510],
                    AF.Exp)
                nc.scalar.activation(E[:nr, 1020:1120], ps[:nr, 1024:1124], AF.Exp)
                # EJ = exp(joint)
                nc.scalar.activation(ej_all[:nr, t * L: (t + 1) * L],
                                     ps[:nr, JCOL: JCOL + L], AF.Exp)

                # s = sum_o E ; rs = 1/s = exp(-ln s)
                s = sml.tile([128, S], F32, tag="s")
                nc.vector.tensor_reduce(
                    s[:nr, 0:102],
                    E[:nr, 0:1020].rearrange("p (z o) -> p z o", o=O),
                    axis=AXL.X, op=ALU.add)
                nc.vector.tensor_reduce(
                    s[:nr, 102:112],
                    E[:nr, 1020:1120].rearrange("p (z o) -> p z o", o=O),
                    axis=AXL.X, op=ALU.add)
                ls = sml.tile([128, S], F32, tag="ls")
                nc.scalar.activation(ls[:nr], s[:nr], AF.Ln)
                rs = sml.tile([128, S], F32, tag="rs")
                nc.scalar.activation(rs[:nr], ls[:nr], AF.Exp, scale=-1.0)

                # P = E * rs (softmax probs)
                P = big.tile([128, S * O], F16, tag="P")
                nc.gpsimd.tensor_mul(
                    P[:nr].rearrange("p (z o) -> p z o", o=O),
                    E[:nr].rearrange("p (z o) -> p z o", o=O),
                    rs[:nr].unsqueeze(2).broadcast_to([nr, S, O]))

                # X = exp(P); zs = sum_o X; lz = ln zs
                X = big.tile([128, S * O], F32, tag="X")
                nc.scalar.activation(X[:nr], P[:nr], AF.Exp)
                zs = sml.tile([128, S], F32, tag="zs")
                nc.vector.tensor_reduce(
                    zs[:nr], X[:nr].rearrange("p (z o) -> p z o", o=O),
                    axis=AXL.X, op=ALU.add)
                lz = sml.tile([128, S], F32, tag="lz")
                nc.scalar.activation(lz[:nr], zs[:nr], AF.Ln)

                # partial sums
                selq_t = big.tile([128, S * O], U8, tag="selq")
                nc.scalar.dma_start(selq_t[:],
                                    selq_d[:, t * S * O: (t + 1) * S * O])
                scr_b = big.tile([128, S * O], BF16, tag="scrb")
                nc.vector.scalar_tensor_tensor(
                    out=scr_b[:nr], in0=P[:nr], scalar=1.0, in1=selq_t[:nr],
                    op0=ALU.mult, op1=ALU.mult,
                    accum_out=strip_pl[:nr, t: t + 1])
                scr_s = sml.tile([128, S], BF16, tag="scrs")
                nc.vector.scalar_tensor_tensor(
                    out=scr_s[:nr], in0=lz[:nr], scalar=1.0,
                    in1=maskq[:nr, t * S: (t + 1) * S],
                    op0=ALU.mult, op1=ALU.mult,
                    accum_out=strip_lz[:nr, t: t + 1])

                # stage exp(q) out (one contiguous transfer per tile)
                deng = nc.sync if t % 2 == 0 else nc.scalar
                deng.dma_start(eq_d[g0: g0 + nr, :], E[:nr, :])

            # ---- joint (element) loss, batched over all tiles ---------------
            sj = med.tile([128, NT], F32, tag="sj")
            nc.vector.tensor_reduce(
                sj[:], ej_all[:].rearrange("p (t l) -> p t l", l=L),
                axis=AXL.X, op=ALU.add)
            lsj = med.tile([128, NT], F32, tag="lsj")
            nc.scalar.activation(lsj[:], sj[:], AF.Ln)
            tmpj = med.tile([128, NT * L], BF16, tag="tmpj")
            ejl = med.tile([128, NT], F32, tag="ejl")
            nc.vector.tensor_mul(tmpj[:], ej_all[:], selj[:])
            nc.vector.tensor_reduce(
                ejl[:], tmpj[:].rearrange("p (t l) -> p t l", l=L),
                axis=AXL.X, op=ALU.add)
            # guard: masked-out rows have selj == 0 -> ejl == 0 -> ln(0);
            # clamp with max(ejl, tiny) to keep ln finite (result masked later)
            ejl2 = med.tile([128, NT], F32, tag="ejl2")
            nc.vector.tensor_scalar_max(ejl2[:], ejl[:], 1e-30)
            lnjl = med.tile([128, NT], F32, tag="lnjl")
            nc.scalar.activation(lnjl[:], ejl2[:], AF.Ln)
            dj = med.tile([128, NT], F32, tag="dj")
            nc.vector.tensor_sub(dj[:], lsj[:], lnjl[:])
            djm = med.tile([128, NT], F32, tag="djm")
            nc.vector.tensor_mul(djm[:], dj[:], maskj[:])

            nc.sync.dma_start(ej_d[:], ej_all[:])
            nc.sync.dma_start(part_d[:, 0:NT], strip_lz[:])
            nc.sync.dma_start(part_d[:, NT: 2 * NT], strip_pl[:])
            nc.sync.dma_start(part_d[:, 2 * NT: 3 * NT], djm[:])

    nc.finalize()
    return nc


_NC_CACHE = None


def _get_program():
    global _NC_CACHE
    if _NC_CACHE is None:
        _NC_CACHE = _build_program()
    return _NC_CACHE


def _pack_inputs(seq, jl, jlm, qm, qmm, pair_w, pair_b, final_w, final_b,
                 value_w, value_b, U):
    """Build the 8 per-core input maps (all host-side numpy)."""
    xT = seq.transpose(2, 0, 1).reshape(H, 2 * S)          # [h, (b,s)]
    xt_h = xT.reshape(HC, 128, 2 * S).transpose(1, 0, 2).reshape(128, HC * 2 * S)
    xt_h = np.ascontiguousarray(xt_h, dtype=np.float32)

    def wpack(w):  # [H, I] -> [128, (hc, i)]
        return np.ascontiguousarray(
            w.reshape(HC, 128, I).transpose(1, 0, 2).reshape(128, HC * I),
            dtype=np.float32)

    wh_h = wpack(pair_w[:H])
    wt_h = wpack(pair_w[H:])
    wv_h = wpack(value_w)
    # U^T: [o, j, i] with j on partitions -> [128, (o, jc, i)]
    ut_h = np.ascontiguousarray(
        U.transpose(0, 2, 1).reshape(O, IC, 128, I).transpose(2, 0, 1, 3)
        .reshape(128, O * IC * I), dtype=np.float32)
    wf_h = np.ascontiguousarray(
        final_w.reshape(IC, 128, L).transpose(1, 0, 2).reshape(128, IC * L),
        dtype=np.float32)
    fb_h = np.ascontiguousarray(final_b.reshape(1, L), dtype=np.float32)
    pb_h = np.ascontiguousarray(pair_b.reshape(IC, 128).T, dtype=np.float32)
    vb_h = np.ascontiguousarray(value_b.reshape(IC, 128).T, dtype=np.float32)

    jl = jl.astype(np.int32)
    qm = qm.astype(np.int32)
    jlm_f = jlm.astype(np.float32)
    qmm_f = qmm.astype(np.float32)

    in_maps = []
    for c in range(NC):
        x0 = c * XC
        hxt_h = np.ascontiguousarray(
            xT.reshape(HC, 128, B, S)[:, :, :, x0: x0 + XC]
            .transpose(1, 0, 2, 3).reshape(128, HC * B * XC), dtype=np.float32)

        # per-row tensors in tile layout [128 partitions, NT * inner]
        selq_h = np.zeros((128, NT, S, O), dtype=np.uint8)
        maskq_h = np.zeros((128, NT, S), dtype=np.float32)
        selj_h = np.zeros((128, NT, L), dtype=np.uint8)
        maskj_h = np.zeros((128, NT), dtype=np.float32)
        for t, (b, r0, nr) in enumerate(TILES):
            rows = np.arange(r0, r0 + nr)
            xl = rows // S
            y = rows % S
            xg = x0 + xl
            lbl_q = qm[b, xg, y]                       # [nr, S] int  (z axis)
            msk_q = qmm_f[b, xg, y]                    # [nr, S]
            onehot = (lbl_q[:, :, None] == np.arange(O)[None, None, :])
            selq_h[:nr, t] = (onehot & (msk_q[:, :, None] > 0)).astype(np.uint8)
            maskq_h[:nr, t] = msk_q
            lbl_j = jl[b, xg, y]                       # [nr] int
            msk_j = jlm_f[b, xg, y]                    # [nr]
            oh_j = (lbl_j[:, None] == np.arange(L)[None, :])
            selj_h[:nr, t] = (oh_j & (msk_j[:, None] > 0)).astype(np.uint8)
            maskj_h[:nr, t] = msk_j

        in_maps.append(dict(
            xt=xt_h, hxt=hxt_h, wh=wh_h, wt=wt_h, wv=wv_h, ut=ut_h, wf=wf_h,
            fb=fb_h, pb=pb_h, vb=vb_h,
            selq=selq_h.reshape(128, NT * S * O),
            maskq=maskq_h.reshape(128, NT * S).astype(ml_dtypes.bfloat16),
            selj=selj_h.reshape(128, NT * L),
            maskj=maskj_h.astype(ml_dtypes.bfloat16),
        ))
    return in_maps


def kernel(seq_encoder_reprs, joint_label_matrix, joint_label_matrix_mask,
           quintuplet_matrix, quintuplet_matrix_mask,
           pair_w, pair_b, final_w, final_b, value_w, value_b, U,
           _want_trace=False):
    seq = np.asarray(seq_encoder_reprs, dtype=np.float32)
    in_maps = _pack_inputs(seq, np.asarray(joint_label_matrix),
                           np.asarray(joint_label_matrix_mask),
                           np.asarray(quintuplet_matrix),
                           np.asarray(quintuplet_matrix_mask),
                           np.asarray(pair_w, dtype=np.float32),
                           np.asarray(pair_b, dtype=np.float32),
                           np.asarray(final_w, dtype=np.float32),
                           np.asarray(final_b, dtype=np.float32),
                           np.asarray(value_w, dtype=np.float32),
                           np.asarray(value_b, dtype=np.float32),
                           np.asarray(U, dtype=np.float32))

    nc = _get_program()
    res = run_bass_kernel_spmd(nc, in_maps, list(range(NC)),
                               trace=_want_trace)
    kernel._last_results = res

    q_score = np.empty((B, S, S, S, O), dtype=np.float32)
    joint_score = np.empty((B, S, S, L), dtype=np.float32)
    q_sum = 0.0
    el_sum = 0.0
    for c in range(NC):
        r = res.results[c]
        x0 = c * XC
        eq = r["eq"].reshape(B, XC, S, S, O)
        q_score[:, x0: x0 + XC] = np.log(eq)
        ej = r["ej"]                                  # [128, NT*L]
        ejr = ej.reshape(128, NT, L)
        for t, (b, r0, nr) in enumerate(TILES):
            rows = np.arange(r0, r0 + nr)
            joint_score[b, x0 + rows // S, rows % S] = np.log(ejr[:nr, t])
        p = r["partials"].astype(np.float64)
        q_sum += p[:, 0:NT].sum() - p[:, NT:2 * NT].sum()
        el_sum += p[:, 2 * NT:3 * NT].sum()

    q_cnt = max(float(np.asarray(quintuplet_matrix_mask).sum()), 1.0)
    el_cnt = max(float(np.asarray(joint_label_matrix_mask).sum()), 1.0)
    loss = np.float32(el_sum / el_cnt + q_sum / q_cnt)
    return loss, joint_score, q_score


kernel._last_exec_ns = None
